# revision 45
# baseline (speedup 1.0000x reference)
"""DynamicMoE (B=4, S=2048, D=1024, E=8, H=4096, top-2) on 8 trn2 cores.

Key observation: the reference loops experts in index order and OVERWRITES
(out = where(w_i>0, y_i, out)), so each token's final output comes from the
single highest-indexed expert of its top-2. Each token therefore needs exactly
one expert MLP, with its input pre-scaled by that expert's softmax score.

Strategy (expert-parallel with host-side routing):
- Host: route in f64, scale+permute tokens by expert, cast x and the expert
  weights to bf16 (error ~4e-3 « 2e-2 tol; halves HBM traffic so the PE is
  the only roofline), pack (expert, token block) parts into 8 cores x NSLOT
  weight slots via a small DP packer. Slots are <=496 tokens so a full slot
  fits one PSUM bank: one matmul per (hi, k), one activation per hi.
- Device (one SPMD program): per slot, a 2-layer MLP in bf16 with tokens on
  the matmul moving dim. Weights stream on the sync DMA ring in big blocks
  (512KB w1 hi-pairs, 1MB w2 di-blocks) in exact consumption order; x,
  biases and outputs ride the scalar ring so they never delay weights.
  fp32 PSUM accumulate, fp32 output.
"""

import numpy as np

_B, _S, _D, _E, _H = 4, 2048, 1024, 8, 4096
_N = _B * _S
_KD = _D // 128   # 8 d-chunks (layer-1 contraction / layer-2 output)
_HI = _H // 128   # 32 h-chunks
_MAXSLOT = 496    # one PSUM bank (512 fp32) minus slack; moving-dim max 512


def _route(x, gate_w, gate_b):
    """Per-token (expert, scale): the higher-indexed of the top-2 experts and
    its softmax score. f64 to track the f32 reference's ordering closely."""
    xf = x.reshape(_N, _D).astype(np.float64)
    logits = xf @ gate_w.astype(np.float64).T + gate_b.astype(np.float64)
    # jax.lax.top_k tie-break: smaller index first -> stable descending sort
    top2 = np.argsort(-logits, axis=1, kind="stable")[:, :2]
    e_sel = top2.max(axis=1)
    m = logits.max(axis=1, keepdims=True)
    p = np.exp(logits - m)
    p /= p.sum(axis=1, keepdims=True)
    scale = p[np.arange(_N), e_sel]
    return e_sel.astype(np.int64), scale.astype(np.float32)


def _pack(counts, slot_sizes_list):
    """Pick a slot structure and assign experts to (core, slot) parts.

    Each candidate is a list of per-core slot sizes [s0, s1, ...]; every core
    runs the same structure, 8 slots of each size exist in total. Feasibility
    via DP over how many slots of each size every expert consumes.
    Returns (slot_sizes, parts) with parts = list of (expert, count, core,
    slot_idx); count <= slot size, zero-padded on device.
    """
    experts = [e for e in range(_E) if counts[e] > 0]

    from itertools import product

    best = None
    for sizes in slot_sizes_list:
        nslot = len(sizes)

        # pipeline model: slots run largest-first; the weight stream (bf16,
        # ~16.9MB/slot at ~270GB/s) is in-order, so a slot can't finish
        # before its weights have streamed in. Minimize modeled finish time.
        desc = sorted(sizes, reverse=True)
        dma_done = 6e3  # startup
        finish = 6e3
        for Ls in desc:
            dma_done += 16.9e6 / 270.0
            finish = max(finish + Ls * 213.0, dma_done)
        score = finish
        if best is not None and score >= best[0]:
            continue

        # options per expert: tuples (k_0..k_{nslot-1}) with sum(k_i*s_i) >= n
        def expert_opts(n):
            opts = [
                t for t in product(range(9), repeat=nslot)
                if sum(k * s for k, s in zip(t, sizes)) >= n
            ]
            return sorted(
                opts,
                key=lambda t: (sum(k * s for k, s in zip(t, sizes)), sum(t)),
            )[:64]

        states = {tuple([0] * nslot): []}
        ok = True
        for e in experts:
            nxt = {}
            for opt in expert_opts(int(counts[e])):
                for st, hist in states.items():
                    ns = tuple(a + b for a, b in zip(st, opt))
                    if all(v <= 8 for v in ns) and ns not in nxt:
                        nxt[ns] = hist + [(e, opt)]
            if not nxt:
                ok = False
                break
            states = nxt
        if not ok:
            continue
        alloc = min(states.values(), key=lambda h: 0)  # any feasible
        best = (score, sizes, alloc)

    assert best is not None, f"no feasible slot structure for counts={counts}"
    _, sizes, alloc = best
    # candidates are generated largest-first; the device processes slots in
    # index order, and largest-first lets the big slot's compute cover the
    # later slots' weight streams
    assert sizes == sorted(sizes, reverse=True), sizes

    # materialize parts: per slot-kind, hand out slot indices core 0..7
    next_core = [0] * len(sizes)
    parts = []
    for e, opt in alloc:
        rem = int(counts[e])
        # fill largest slots first
        fill_order = sorted(range(len(sizes)), key=lambda i: -sizes[i])
        for i in fill_order:
            for _ in range(opt[i]):
                take = max(0, min(rem, sizes[i]))
                core = next_core[i]
                next_core[i] += 1
                parts.append((e, take, core, i))
                rem -= take
        assert rem <= 0
    return list(sizes), parts


_PROG_CACHE = {}


def _build_program(slot_sizes):
    """One SPMD Bass program for all 8 cores, parameterized by slot sizes."""
    import concourse.tile as tile
    from concourse import bacc, mybir

    key = tuple(slot_sizes)
    if key in _PROG_CACHE:
        return _PROG_CACHE[key]

    F32 = mybir.dt.float32
    BF16 = mybir.dt.bfloat16
    CAP = sum(slot_sizes)
    nslot = len(slot_sizes)
    NP = _HI // 2  # w1 hi-pairs

    nc = bacc.Bacc("TRN2", target_bir_lowering=False, debug=False, num_devices=8)
    # x stored per slot, contiguous per partition, so each slot's x is one
    # straight 128 x (KD*Ls*2B) DMA — strided x slices measured ~4x slower
    # per byte during the startup ramp (480B descriptors)
    xtd = [
        nc.dram_tensor(f"xt_{s}", [128, _KD, slot_sizes[s]], BF16,
                       kind="ExternalInput").ap()
        for s in range(nslot)
    ]
    # w1 packed as hi-pairs: [pair, part, j(2), k, f]
    w1d = [
        nc.dram_tensor(f"w1_{s}", [NP, 128, 2, _KD, 128], BF16,
                       kind="ExternalInput").ap()
        for s in range(nslot)
    ]
    w2d = [
        nc.dram_tensor(f"w2_{s}", [_KD, 128, _HI, 128], BF16,
                       kind="ExternalInput").ap()
        for s in range(nslot)
    ]
    b1d = [
        nc.dram_tensor(f"b1_{s}", [128, _HI], F32, kind="ExternalInput").ap()
        for s in range(nslot)
    ]
    b2d = [
        nc.dram_tensor(f"b2_{s}", [128, _KD], F32, kind="ExternalInput").ap()
        for s in range(nslot)
    ]
    outT = nc.dram_tensor("outT", [_KD, 128, CAP], F32, kind="ExternalOutput").ap()

    Relu = mybir.ActivationFunctionType.Relu
    Ident = mybir.ActivationFunctionType.Identity

    offs = [0]
    for Ls in slot_sizes:
        offs.append(offs[-1] + Ls)

    with tile.TileContext(nc, pool_alloc_mode="queue") as tc:
        with tc.tile_pool(name="xp", bufs=2) as xp, \
             tc.tile_pool(name="w1p", bufs=16) as w1p, \
             tc.tile_pool(name="w1f", bufs=2) as w1f, \
             tc.tile_pool(name="w2p", bufs=5) as w2p, \
             tc.tile_pool(name="h1p", bufs=2) as h1p, \
             tc.tile_pool(name="cp", bufs=4) as cp, \
             tc.tile_pool(name="op", bufs=4) as op, \
             tc.tile_pool(name="ps1", bufs=4, space="PSUM") as ps1, \
             tc.tile_pool(name="ps2", bufs=4, space="PSUM") as ps2:

            slot_pre = {}

            def emit_slot_prefetch(s):
                """Queue slot s's x + b2 on the scalar DMA ring and the
                first w1 pair + b1 on the sync (weights) ring. The two
                hardware rings run concurrently, so x never delays weight
                streaming. For slot 0 (kernel startup) the first w1 pair is
                halved and the first odd pairs are pre-dispatched on the
                scalar ring so both rings feed the ramp."""
                Ls = slot_sizes[s]
                xc = xp.tile([128, _KD, Ls], BF16, tag="x")
                w1_0 = w1f.tile([128, 2, _KD, 128], BF16, tag="w1first")
                b1_sb = cp.tile([128, _HI], F32, tag="b1")
                b2_sb = cp.tile([128, _KD], F32, tag="b2")
                if s == 0:
                    # single contiguous x load: a later PE start with zero
                    # interruptions beats an earlier start with trickle gaps
                    # (each gap also resets the 3us PE clock ramp)
                    nc.sync.dma_start(w1_0[:, 0, :, :], w1d[s][0, :, 0, :, :])
                    nc.scalar.dma_start(xc[:], xtd[s][:])
                    nc.sync.dma_start(b1_sb[:], b1d[s][:])
                    nc.sync.dma_start(w1_0[:, 1, :, :], w1d[s][0, :, 1, :, :])
                    nc.scalar.dma_start(b2_sb[:], b2d[s][:])
                    # dispatch the first odd pairs on the scalar ring NOW —
                    # emitted inside the L1 loop they would sit behind the
                    # first ACTIVATE in the scalar stream, which stalls on
                    # the startup matmul group and delays them ~10us
                    pre_pairs = {}
                    for pi in (1, 3):
                        wt = w1p.tile([128, 2, _KD, 128], BF16, tag="w1")
                        nc.scalar.dma_start(wt[:], w1d[s][pi])
                        pre_pairs[pi] = wt
                else:
                    nc.scalar.dma_start(xc[:], xtd[s][:])
                    nc.sync.dma_start(w1_0[:], w1d[s][0])
                    nc.scalar.dma_start(b1_sb[:], b1d[s][:])
                    nc.scalar.dma_start(b2_sb[:], b2d[s][:])
                    pre_pairs = {}
                slot_pre[s] = (xc, w1_0, b1_sb, b2_sb, pre_pairs)

            emit_slot_prefetch(0)
            for s, Ls in enumerate(slot_sizes):
                off = offs[s]
                x_sb, w1_first, b1_sb, b2_sb, pre_pairs = slot_pre.pop(s)

                h1_sb = h1p.tile([128, _HI, Ls], BF16, tag="h1")
                w2_sb = {}

                def emit_w2(di):
                    wt = w2p.tile([128, _HI, 128], BF16, tag="w2")
                    nc.sync.dma_start(wt[:], w2d[s][di])
                    w2_sb[di] = wt

                w1_tiles = {0: w1_first}
                w1_tiles.update(pre_pairs)
                l1_iter = [(hi, 0, Ls) for hi in range(_HI)]
                for hi, ca, cb in l1_iter:
                    pi = hi // 2
                    if pi not in w1_tiles:
                        wt = w1p.tile([128, 2, _KD, 128], BF16, tag="w1")
                        nc.sync.dma_start(wt[:], w1d[s][pi])
                        w1_tiles[pi] = wt
                    if hi == _HI - 6:
                        emit_w2(0)      # w2 di=0 streams during L1's tail
                    if hi == _HI - 3:
                        emit_w2(1)
                    if hi == _HI - 1:
                        emit_w2(2)
                    w1_sb = w1_tiles[pi][:, hi % 2]
                    ps = ps1.tile([128, cb - ca], F32, tag="ps1")
                    for k in range(_KD):
                        nc.tensor.matmul(
                            ps[:], w1_sb[:, k, :], x_sb[:, k, ca:cb],
                            start=(k == 0), stop=(k == _KD - 1),
                        )
                    nc.scalar.activation(
                        h1_sb[:, hi, ca:cb], ps[:], Relu,
                        bias=b1_sb[:, hi:hi + 1],
                    )

                for di in range(_KD):
                    if di + 3 < _KD:
                        emit_w2(di + 3)
                    if di == _KD - 3 and s + 1 < nslot:
                        # this slot's weights are nearly all queued; prefetch
                        # the next slot's x/biases/first-w1 behind them
                        emit_slot_prefetch(s + 1)
                    if s == nslot - 1 and di == _KD - 1 and Ls > 128:
                        # final di of the last slot: column-split so earlier
                        # pieces' activation + out-DMA overlap later pieces'
                        # matmuls, shortening the drain tail
                        q = Ls // 4
                        splits = ((0, q), (q, 2 * q), (2 * q, 3 * q),
                                  (3 * q, Ls))
                        for ca, cb in splits:
                            ps = ps2.tile([128, cb - ca], F32, tag="ps2")
                            for hi in range(_HI):
                                nc.tensor.matmul(
                                    ps[:], w2_sb[di][:, hi, :],
                                    h1_sb[:, hi, ca:cb],
                                    start=(hi == 0), stop=(hi == _HI - 1),
                                )
                            ob = op.tile([128, cb - ca], F32, tag="ob")
                            nc.scalar.activation(
                                ob[:], ps[:], Ident, bias=b2_sb[:, di:di + 1],
                            )
                            nc.scalar.dma_start(
                                outT[di, :, off + ca:off + cb], ob[:]
                            )
                    else:
                        ps = ps2.tile([128, Ls], F32, tag="ps2")
                        for hi in range(_HI):
                            nc.tensor.matmul(
                                ps[:], w2_sb[di][:, hi, :], h1_sb[:, hi, :],
                                start=(hi == 0), stop=(hi == _HI - 1),
                            )
                        ob = op.tile([128, Ls], F32, tag="ob")
                        nc.scalar.activation(
                            ob[:], ps[:], Ident, bias=b2_sb[:, di:di + 1],
                        )
                        nc.scalar.dma_start(outT[di, :, off:off + Ls], ob[:])

    nc.compile()
    _PROG_CACHE[key] = nc
    return nc


def _run(x, gate_w, gate_b, w1, b1, w2, b2, trace=False, trace_cores=None):
    import ml_dtypes
    from concourse import bass_utils

    BF = ml_dtypes.bfloat16

    e_sel, scale = _route(x, gate_w, gate_b)
    counts = np.bincount(e_sel, minlength=_E)

    # candidate structures (all slots <= _MAXSLOT so a slot spans one PSUM
    # bank): 2-slot, 3-slot grids, near-equal fine triples (every slot large
    # enough that its PE time covers its own weight stream), and fallbacks
    cands = []
    for A in range(256, _MAXSLOT + 1, 16):
        for Bv in range(256, A + 1, 16):
            cands.append([A, Bv])
    for a in range(384, _MAXSLOT + 1, 32):
        for b in range(160, a + 1, 32):
            for c in range(96, b + 1, 32):
                if 1024 <= a + b + c <= 1120:
                    cands.append([a, b, c])
    for tot in range(1024, 1073, 8):
        for a in range(tot // 3, _MAXSLOT + 1, 8):
            for b in range(max(264, tot - a - _MAXSLOT), a + 1, 8):
                c = tot - a - b
                if 264 <= c <= b:
                    cands.append([a, b, c])
    cands.append([496, 496, 496])
    cands.append([496, 496, 496, 496])
    slot_sizes, parts = _pack(counts, cands)
    CAP = sum(slot_sizes)

    # token ids per expert in sorted order
    order = np.argsort(e_sel, kind="stable")
    starts = np.zeros(_E + 1, np.int64)
    np.cumsum(counts, out=starts[1:])
    consumed = [0] * _E

    # slot offsets within a core's token axis
    offs = np.zeros(len(slot_sizes) + 1, np.int64)
    np.cumsum(slot_sizes, out=offs[1:])

    xs = (x.reshape(_N, _D) * scale[:, None]).astype(BF)

    # prearranged weights, one contiguous block per (expert, hi-pair):
    # W1L[e, pair, p, j, k, f] = w1[e, (2*pair+j)*128+f, k*128+p]
    W1L = np.ascontiguousarray(
        w1.reshape(_E, _HI // 2, 2, 128, _KD, 128)
        .transpose(0, 1, 5, 2, 4, 3).astype(BF)
    )
    # W2L[e, di, p, hi, f] = w2[e, di*128+f, hi*128+p]
    W2L = np.ascontiguousarray(
        w2.reshape(_E, _KD, 128, _HI, 128).transpose(0, 1, 4, 3, 2).astype(BF)
    )
    B1L = np.ascontiguousarray(b1.reshape(_E, _HI, 128).transpose(0, 2, 1))
    B2L = np.ascontiguousarray(b2.reshape(_E, _KD, 128).transpose(0, 2, 1))

    slot_expert = [[0] * len(slot_sizes) for _ in range(8)]
    tok_of = np.full((8, CAP), -1, np.int64)
    for (e, cnt, core, si) in parts:
        lo = starts[e] + consumed[e]
        consumed[e] += cnt
        toks = order[lo:lo + cnt]
        tok_of[core, offs[si]:offs[si] + cnt] = toks
        slot_expert[core][si] = e

    in_maps = []
    for core in range(8):
        cols = tok_of[core]
        xsel = np.zeros((CAP, _D), BF)
        valid = cols >= 0
        xsel[valid] = xs[cols[valid]]
        XL = xsel.reshape(CAP, _KD, 128).transpose(2, 1, 0)
        m = {}
        for si in range(len(slot_sizes)):
            e = slot_expert[core][si]
            # per-slot x, contiguous [128, KD, Ls] block
            m[f"xt_{si}"] = np.ascontiguousarray(
                XL[:, :, offs[si]:offs[si + 1]]
            )
            m[f"w1_{si}"] = W1L[e]
            m[f"w2_{si}"] = W2L[e]
            m[f"b1_{si}"] = B1L[e]
            m[f"b2_{si}"] = B2L[e]
        in_maps.append(m)

    nc = _build_program(slot_sizes)
    kw = {}
    if trace:
        kw["trace"] = True
        if trace_cores is not None:
            kw["trace_cores"] = trace_cores
    try:
        res = bass_utils.run_bass_kernel_spmd(
            nc, in_maps, core_ids=list(range(8)), **kw
        )
    except Exception:
        # one retry for transient device faults
        import time as _time
        _time.sleep(2.0)
        res = bass_utils.run_bass_kernel_spmd(
            nc, in_maps, core_ids=list(range(8)), **kw
        )

    out = np.zeros((_N, _D), np.float32)
    for core in range(8):
        cols = tok_of[core]
        valid = cols >= 0
        oc = res.results[core]["outT"]  # [KD, 128, CAP]
        ovals = np.asarray(oc, dtype=np.float32).transpose(2, 0, 1).reshape(CAP, _D)
        out[cols[valid]] = ovals[valid]
    return out.reshape(_B, _S, _D), res


def kernel(x, gate_w, gate_b, w1, b1, w2, b2):
    x = np.ascontiguousarray(np.asarray(x, dtype=np.float32))
    gate_w = np.asarray(gate_w, dtype=np.float32)
    gate_b = np.asarray(gate_b, dtype=np.float32)
    w1 = np.ascontiguousarray(np.asarray(w1, dtype=np.float32))
    b1 = np.asarray(b1, dtype=np.float32)
    w2 = np.ascontiguousarray(np.asarray(w2, dtype=np.float32))
    b2 = np.asarray(b2, dtype=np.float32)
    out, _ = _run(x, gate_w, gate_b, w1, b1, w2, b2)
    return out
